# revision 29
# baseline (speedup 1.0000x reference)
"""Trainium2 Bass kernel for nn_BaselineModel_47682726921062.

Model: token embedding lookup -> input projection -> 512-step tanh RNN
-> softmax over the hidden dim. Output [64, 512, 512] = softmax(h, axis=1)
with h[b, :, t] the hidden state after step t.

Strategy: data-parallel over batch across 8 NeuronCores (8 examples/core),
weights replicated, zero collectives. The 512-step recurrence is a serial
latency chain; everything else is folded under it:

  - per-step chain is PE -> ACT only: the input projection xp_t (+bias)
    is accumulated INTO the recurrence PSUM banks ahead of time by PE
    matmuls (bias via a K=1 matmul against a ones row), so the W_hh
    matmuls accumulate on top (start=False) and tanh reads PSUM directly.
    No per-step VectorE add, no separate projection phase.
  - PSUM banks hold GB=16 steps each and rotate (bufs=3): bank g+2 is
    preloaded by xp matmuls interleaved into group g's step slots, off
    the critical path.
  - embedding rows are gathered *and transposed* in one SWDGE dma_gather
    per 64-step block (fp16, elem 1KB); all 8 gathers are issued up
    front and stream ahead of the projection matmuls.
  - fp16 everywhere bf16 was: same speed class, ~8x the mantissa
    precision through the 512-step chain.
  - softmax runs interleaved under the recurrence: one small exp (ACT)
    per step fills ACT's idle window inside the chain period; partition
    sums via ones-matmul on PE, fast reciprocal + normalize on VectorE,
    and DMA out fire once per 64-step block, deferred two steps so they
    never block the tanh chain.

TimelineSim (single-core model): 389.5 us vs 712.7 us for the previous
bf16 three-phase kernel. The 512-step chain runs at ~760 ns/step:
ACT tanh ~270 (185 of it SBUF-access bubble) + two sem hops + 16 PE
matmuls ~100 + 173 PE->sem pipeline drain, all architectural floors.
start=True on a matmul marks the whole 2KB PSUM bank pending-zero, so
each bank epoch uses exactly one start=True (first bias matmul); every
later matmul (xp, W_hh) must use start=False to accumulate.
"""

import sys

if "/opt/trn_rl_repo" not in sys.path:
    sys.path.insert(0, "/opt/trn_rl_repo")

import numpy as np

BATCH, SEQ, VOCAB, DIM = 64, 512, 32000, 512
NCORES = 8
BC = BATCH // NCORES          # 8 examples per core
P = 128
KC = DIM // P                 # 4 chunks of 128
NIDX = SEQ * BC               # 4096 gathered rows per core
NBLK = 16                     # gather blocks of 256 (t,b) columns
BLK = NIDX // NBLK            # 512
GB = 16                       # timesteps per PSUM group/bank
NG = SEQ // GB                # 32 groups
GCOLS = GB * BC               # 128 moving columns per xp matmul
TB = 8                        # softmax/output t-blocks
TBS = SEQ // TB               # 64 timesteps per block

TRACE = False
LAST_RESULT = None
REC_BUFS = 3
EXPC = 1                      # exp chunk size (timesteps per ACT exp instr)
SMAX = 1                      # 0 disables softmax (model experiments only)
DRIP = 2                      # max preload/softmax PE ops issued per step slot
DUMP = 0                      # 1: also dump hT_all to dram (debug only)
WARM = 0                      # extra PE filler matmuls per step (p-state)

_cache = {}


def _build():
    import concourse.mybir as mybir
    import concourse.tile as tile
    from concourse import bacc

    f32 = mybir.dt.float32
    f16 = mybir.dt.float16

    nc = bacc.Bacc("TRN2")

    emb = nc.dram_tensor("emb", [VOCAB, DIM], f16, kind="ExternalInput")
    idx = nc.dram_tensor("idx", [P, NIDX // 16], mybir.dt.int16, kind="ExternalInput")
    wih = nc.dram_tensor("wih", [DIM, DIM], f16, kind="ExternalInput")   # W_ih.T
    whh = nc.dram_tensor("whh", [DIM, DIM], f16, kind="ExternalInput")   # W_hh.T
    bias = nc.dram_tensor("bias", [1, DIM], f16, kind="ExternalInput")   # b_ih+b_hh
    ones = nc.dram_tensor("ones", [P, P], f16, kind="ExternalInput")
    out = nc.dram_tensor("out", [BC, DIM, SEQ], f32, kind="ExternalOutput")
    hdump = (
        nc.dram_tensor("hdump", [P, SEQ, KC, BC], f16, kind="ExternalOutput")
        if DUMP else None
    )

    with tile.TileContext(nc) as tc:
        with (
            tc.tile_pool(name="consts", bufs=1) as consts,
            tc.tile_pool(name="xe", bufs=1) as xe_pool,
            tc.tile_pool(name="h", bufs=1) as h_pool,
            tc.tile_pool(name="sm", bufs=1) as sm_pool,
            tc.tile_pool(name="expb", bufs=2) as exp_pool,
            tc.tile_pool(name="stage", bufs=2) as stage_pool,
            tc.tile_pool(name="rps", bufs=REC_BUFS, space="PSUM") as rps,
            tc.tile_pool(name="sps", bufs=2, space="PSUM") as sps,
        ):
            idx_sb = consts.tile([P, NIDX // 16], mybir.dt.int16)
            nc.sync.dma_start(idx_sb[:], idx[:])
            wih_sb = consts.tile([P, KC, DIM], f16)
            nc.sync.dma_start(wih_sb[:], wih.rearrange("(kc p) m -> p kc m", p=P))
            whh_sb = consts.tile([P, KC, DIM], f16)
            nc.sync.dma_start(whh_sb[:], whh.rearrange("(kc p) m -> p kc m", p=P))
            bias_sb = consts.tile([1, DIM], f16)
            nc.sync.dma_start(bias_sb[:], bias[:])
            ones_sb = consts.tile([P, P], f16)
            nc.sync.dma_start(ones_sb[:], ones[:])

            xe_all = xe_pool.tile([P, NBLK, KC, BLK], f16)   # 32 KB/partition
            hT_all = h_pool.tile([P, SEQ, KC, BC], f16)      # 32 KB/partition
            recip_sb = sm_pool.tile([P, SEQ, BC], f32)       # 16 KB/partition

            # All embedding gathers up front; they stream ahead of the
            # projection matmuls that consume them.
            for nb in range(NBLK):
                nc.gpsimd.dma_gather(
                    xe_all[:, nb], emb[:],
                    idx_sb[:, nb * (BLK // 16) : (nb + 1) * (BLK // 16)],
                    num_idxs=BLK, num_idxs_reg=BLK, elem_size=DIM,
                    transpose=True,
                )

            rec_tiles = {}

            def xp_ops(g):
                """Matmuls preloading bank g with bias + W_ih @ xe for its
                16 steps, as closures to spread across earlier step slots."""
                ps = rps.tile([P, KC, GB, BC], f32, tag="rec")
                rec_tiles[g] = ps
                ops = []
                # start=True only on the first write: it marks the whole 2KB
                # bank pending-zero, so the first writer of each byte
                # overwrites and every later matmul accumulates.
                for mc in range(KC):
                    ops.append(lambda mc=mc, ps=ps: nc.tensor.matmul(
                        ps[:, mc].rearrange("p g b -> p (g b)"),
                        bias_sb[0:1, mc * P : (mc + 1) * P],
                        ones_sb[0:1, 0:GCOLS],
                        start=(mc == 0), stop=False, skip_group_check=True,
                    ))
                for kc in range(KC):
                    for mc in range(KC):
                        ops.append(lambda kc=kc, mc=mc, ps=ps: nc.tensor.matmul(
                            ps[:, mc].rearrange("p g b -> p (g b)"),
                            wih_sb[:, kc, mc * P : (mc + 1) * P],
                            xe_all[:, g // 2, kc,
                                   (g % 2) * GCOLS : (g % 2 + 1) * GCOLS],
                            start=False, stop=False, skip_group_check=True,
                        ))
                return ops

            ex_tiles = {}

            def exp_chunk(tb, t0, n):
                """Exp of steps [t0, t0+n) into block tb's ex tile — sized to
                fit ACT's idle window between consecutive tanhs."""
                if tb not in ex_tiles:
                    ex_tiles[tb] = exp_pool.tile(
                        [P, TBS, KC, BC], f16, tag="ex", name=f"ex{tb}"
                    )
                ex = ex_tiles[tb]
                lo = t0 - tb * TBS
                nc.scalar.activation(
                    ex[:, lo : lo + n, :, :], hT_all[:, t0 : t0 + n, :, :],
                    mybir.ActivationFunctionType.Exp,
                )

            def softmax_block(tb):
                """Closure for t-block tb's post-exp softmax (PE sums, DVE
                reciprocal+normalize, DMA out), deferred off the tanh chain."""
                tsl = slice(tb * TBS, (tb + 1) * TBS)
                ex = ex_tiles.pop(tb)
                sp = sps.tile([P, TBS, BC], f32, tag="sum")
                st = stage_pool.tile([P, KC, BC, TBS], f32, tag="st")

                def pe_and_rest():
                    for c in range(KC):
                        nc.tensor.matmul(
                            sp[:], ones_sb[:], ex[:, :, c, :],
                            start=(c == 0), stop=(c == KC - 1),
                        )
                    nc.vector.reciprocal_approx_fast(recip_sb[:, tsl, :], sp[:])
                    for c in range(KC):
                        nc.vector.tensor_tensor(
                            st[:, c].rearrange("p b t -> p t b"),
                            ex[:, :, c, :],
                            recip_sb[:, tsl, :],
                            mybir.AluOpType.mult,
                        )
                    for c in range(KC):
                        nc.sync.dma_start(
                            out[:, c * P : (c + 1) * P, tsl].rearrange(
                                "b p t -> p b t"
                            ),
                            st[:, c],
                        )

                return pe_and_rest

            with nc.named_scope("recurrence"):
                warm_ps = (
                    sps.tile([P, BC], f32, tag="warm", name="warm_ps")
                    if WARM else None
                )
                pending_pe = []          # closures to drip into step slots
                for op in xp_ops(0) + xp_ops(1):
                    op()
                pending_pe.extend(xp_ops(2))
                deferred = []            # (due_step, closure)

                for t in range(SEQ):
                    g, s = t // GB, t % GB
                    ps = rec_tiles[g]
                    if t > 0:
                        for kc in range(KC):
                            for mc in range(KC):
                                nc.tensor.matmul(
                                    ps[:, mc, s, :],
                                    whh_sb[:, kc, mc * P : (mc + 1) * P],
                                    hT_all[:, t - 1, kc, :],
                                    start=False, stop=(kc == KC - 1),
                                    skip_group_check=True,
                                )
                    # drip preload/softmax work into this slot
                    while deferred and deferred[0][0] <= t:
                        pending_pe.append(deferred.pop(0)[1])
                    n_issue = DRIP if len(pending_pe) >= GB - s else 1
                    for _ in range(n_issue):
                        if pending_pe:
                            pending_pe.pop(0)()
                    for _ in range(WARM):
                        nc.tensor.matmul(
                            warm_ps[:], ones_sb[:], ones_sb[:, 0:BC],
                            start=True, stop=True, skip_group_check=True,
                        )
                    nc.scalar.activation(
                        hT_all[:, t, :, :], ps[:, :, s, :],
                        mybir.ActivationFunctionType.Tanh,
                    )
                    if s == GB - 1 and g + 3 < NG:
                        pending_pe.extend(xp_ops(g + 3))
                    if SMAX and (t + 1) % EXPC == 0:
                        exp_chunk(t // TBS, t + 1 - EXPC, EXPC)
                    if SMAX and (t + 1) % TBS == 0:
                        pe_rest = softmax_block(t // TBS)
                        deferred.append((min(t + 2, SEQ - 1), pe_rest))

                while deferred:
                    pending_pe.append(deferred.pop(0)[1])
                for op in pending_pe:
                    op()
                if DUMP:
                    nc.sync.dma_start(hdump[:], hT_all[:])

    nc.compile()
    return nc


def _prep_core_inputs(x_core, shared):
    flat = np.ascontiguousarray(x_core.T).reshape(-1).astype(np.int16)  # j = t*8+b
    idx = np.zeros((P, NIDX // 16), np.int16)
    for nb in range(NBLK):
        blk = flat[nb * BLK : (nb + 1) * BLK].reshape(BLK // 16, 16).T  # [16, 32]
        idx[:, nb * (BLK // 16) : (nb + 1) * (BLK // 16)] = np.tile(
            blk, (P // 16, 1)
        )
    m = dict(shared)
    m["idx"] = idx
    return m


def _shared_inputs(emb, W_ih, W_hh, b_ih, b_hh):
    return {
        "emb": np.ascontiguousarray(emb).astype(np.float16),
        "wih": np.ascontiguousarray(W_ih.T).astype(np.float16),
        "whh": np.ascontiguousarray(W_hh.T).astype(np.float16),
        "bias": np.ascontiguousarray((b_ih + b_hh).reshape(1, DIM)).astype(np.float16),
        "ones": np.ones((P, P), np.float16),
    }


def kernel(x, emb, W_ih, W_hh, b_ih, b_hh):
    global LAST_RESULT
    from concourse.bass_utils import run_bass_kernel_spmd

    x = np.asarray(x)
    emb = np.asarray(emb, dtype=np.float32)
    W_ih = np.asarray(W_ih, dtype=np.float32)
    W_hh = np.asarray(W_hh, dtype=np.float32)
    b_ih = np.asarray(b_ih, dtype=np.float32)
    b_hh = np.asarray(b_hh, dtype=np.float32)

    if "nc" not in _cache:
        _cache["nc"] = _build()
    nc = _cache["nc"]

    shared = _shared_inputs(emb, W_ih, W_hh, b_ih, b_hh)
    in_maps = [
        _prep_core_inputs(x[c * BC : (c + 1) * BC], shared) for c in range(NCORES)
    ]
    res = run_bass_kernel_spmd(
        nc, in_maps, core_ids=list(range(NCORES)), trace=TRACE,
        **({"stitch_traces": True} if TRACE else {}),
    )
    LAST_RESULT = res
    return np.concatenate([res.results[c]["out"] for c in range(NCORES)], axis=0)
